# revision 20
# baseline (speedup 1.0000x reference)
"""Block-diagonal cross-attention + MLP for trn2, 8-core data-parallel.

v3: graphs bin-packed in pairs into 128-row blocks (NB blocks/core).
Cross-graph + padding masking is folded into the score matmul via two
extra contraction rows (graph-code sigma in {+1,-1}): score' = q.k
- 25*(1 - sigma_q*sigma_k), so cross-graph pairs get -50 and padded
columns -25 -> exp ~ 0.  Only exp(SS) is computed on ACT; E^T comes
from a DMA transpose (ST = SS^T exactly).  V-matmul eviction fuses
normalize (divide by the mask-column rowsum) and the +x residual via
scalar_tensor_tensor on DVE/Pool.  er is DMA-transposed straight into
the feature-major MLP input.  MLP uses block-diagonal [128,128]
weights to do both sides at once; residual+bias fused into the final
eviction.  dtypes: fp16 images/weights everywhere except E/V (bf16,
exp can reach e^46 which overflows fp16).
Output: [128, NB*128] fp16 per core; host scatters per graph.
"""

from contextlib import ExitStack

import numpy as np

N_NODES = 8192
D = 64
G = 128
N_CORES = 8
BCAP = 128                  # rows per block
CD = D + 2                  # contraction rows incl. bias rows
VW = D + 1                  # v image width incl. mask column
NEG = 25.0                  # pad bias; cross-graph pairs get -2*NEG

_PROGRAM_CACHE = {}


def _build_program(nb, mlp_ch):
    import concourse.bass as bass
    import concourse.tile as tile
    from concourse import bacc, mybir

    fp32 = mybir.dt.float32
    fp16 = mybir.dt.float16
    bf16 = mybir.dt.bfloat16
    rows = nb * BCAP
    nsb = (nb + 1) // 2           # superblocks of up to 2 blocks
    nc = bacc.Bacc("TRN2", target_bir_lowering=False, debug=False)

    xsT = nc.declare_dram_parameter("xsT", [CD, rows], fp16, isOutput=False)
    xtT = nc.declare_dram_parameter("xtT", [CD, rows], fp16, isOutput=False)
    vs = nc.declare_dram_parameter("vs", [BCAP, nb * VW], bf16, isOutput=False)
    vt = nc.declare_dram_parameter("vt", [BCAP, nb * VW], bf16, isOutput=False)
    vres = nc.declare_dram_parameter("vres", [BCAP, nb * 2 * D], fp16,
                                     isOutput=False)
    w1bd = nc.declare_dram_parameter("w1bd", [2 * D, 2 * D], fp16, isOutput=False)
    b1bd = nc.declare_dram_parameter("b1bd", [2 * D, 1], fp32, isOutput=False)
    w2bd = nc.declare_dram_parameter("w2bd", [2 * D, 2 * D], fp16, isOutput=False)
    b2bd = nc.declare_dram_parameter("b2bd", [2 * D, 1], fp32, isOutput=False)
    idh = nc.declare_dram_parameter("idh", [BCAP, BCAP], fp16, isOutput=False)
    idb = nc.declare_dram_parameter("idb", [BCAP, BCAP], bf16, isOutput=False)
    outp = nc.declare_dram_parameter("outp", [2 * D, rows], fp16, isOutput=True)

    AF = mybir.ActivationFunctionType
    ALU = mybir.AluOpType

    with tile.TileContext(nc) as tc, ExitStack() as ctx:
        singles = ctx.enter_context(tc.tile_pool(name="singles", bufs=1))
        epool = ctx.enter_context(tc.tile_pool(name="epool", bufs=3))
        tpool = ctx.enter_context(tc.tile_pool(name="tpool", bufs=3))
        rpool = ctx.enter_context(tc.tile_pool(name="rpool", bufs=3))

        sb_xsT = singles.tile([CD, rows], fp16, tag="xsT")
        sb_xtT = singles.tile([CD, rows], fp16, tag="xtT")
        sb_vs = singles.tile([BCAP, nb * VW], bf16, tag="vs")
        sb_vt = singles.tile([BCAP, nb * VW], bf16, tag="vt")
        sb_vres = singles.tile([BCAP, nb * 2 * D], fp16, tag="vres")
        sb_w1 = singles.tile([2 * D, 2 * D], fp16, tag="w1")
        sb_b1 = singles.tile([2 * D, 1], fp32, tag="b1")
        sb_w2 = singles.tile([2 * D, 2 * D], fp16, tag="w2")
        sb_b2 = singles.tile([2 * D, 1], fp32, tag="b2")
        sb_eT = singles.tile([2 * D, rows], fp16, tag="eT")
        sb_h = singles.tile([2 * D, rows], fp16, tag="h")
        sb_out = singles.tile([2 * D, rows], fp16, tag="out")
        sb_idh = singles.tile([BCAP, BCAP], fp16, tag="idh")
        sb_idb = singles.tile([BCAP, BCAP], bf16, tag="idb")

        warm = singles.tile([BCAP, 1], fp32, tag="warm")
        nc.vector.memset(warm, 0.0)
        nc.scalar.activation(out=warm, in_=warm, func=AF.Exp)
        sp = 2 * BCAP
        nc.sync.dma_start(out=sb_xsT[:, 0:sp], in_=xsT[:, 0:sp])
        nc.sync.dma_start(out=sb_xtT[:, 0:sp], in_=xtT[:, 0:sp])
        nc.sync.dma_start(out=sb_xsT[:, sp:], in_=xsT[:, sp:])
        nc.sync.dma_start(out=sb_xtT[:, sp:], in_=xtT[:, sp:])
        nc.sync.dma_start(out=sb_vs, in_=vs[:, :])
        nc.sync.dma_start(out=sb_vt, in_=vt[:, :])
        nc.sync.dma_start(out=sb_vres, in_=vres[:, :])
        nc.sync.dma_start(out=sb_w1, in_=w1bd[:, :])
        nc.sync.dma_start(out=sb_b1, in_=b1bd[:, :])
        nc.sync.dma_start(out=sb_w2, in_=w2bd[:, :])
        nc.sync.dma_start(out=sb_b2, in_=b2bd[:, :])
        nc.sync.dma_start(out=sb_idh, in_=idh[:, :])
        nc.sync.dma_start(out=sb_idb, in_=idb[:, :])

        with tc.tile_pool(name="ps_sc", bufs=3, space="PSUM") as ps_sc, \
             tc.tile_pool(name="ps_ot", bufs=3, space="PSUM") as ps_ot, \
             tc.tile_pool(name="ps_m", bufs=2, space="PSUM") as ps_m:
            for s in range(nsb):
                blo = 2 * s
                g = min(2, nb - blo)
                # scores for g blocks, BOTH orientations [SS_j|ST_j] in one
                # psum bank -> one exp; ST computed directly on PE so no
                # E^T transpose is ever needed
                sc = ps_sc.tile([BCAP, g * 2 * BCAP], fp32, tag="sc")
                for j in range(g):
                    c = (blo + j) * BCAP
                    q = j * 2 * BCAP
                    nc.tensor.matmul(sc[:, q:q + BCAP],
                                     sb_xsT[:, c:c + BCAP],
                                     sb_xtT[:, c:c + BCAP],
                                     start=True, stop=True)
                    nc.tensor.matmul(sc[:, q + BCAP:q + 2 * BCAP],
                                     sb_xtT[:, c:c + BCAP],
                                     sb_xsT[:, c:c + BCAP],
                                     start=True, stop=True)
                et = epool.tile([BCAP, g * 2 * BCAP], bf16, tag="E")
                nc.scalar.activation(out=et, in_=sc, func=AF.Exp)

                if True:
                    j0 = 0
                    o = ps_ot.tile([BCAP, g * 2 * VW], fp32, tag="ot")
                    for j in range(j0, j0 + g):
                        b = blo + j
                        va = b * VW
                        q = (j - j0) * 2 * VW
                        e_ss = et[:, j * 2 * BCAP:j * 2 * BCAP + BCAP]
                        e_st = et[:, j * 2 * BCAP + BCAP:(j + 1) * 2 * BCAP]
                        nc.tensor.matmul(o[:, q:q + VW], e_st,
                                         sb_vt[:, va:va + VW],
                                         start=True, stop=True)
                        nc.tensor.matmul(o[:, q + VW:q + 2 * VW], e_ss,
                                         sb_vs[:, va:va + VW],
                                         start=True, stop=True)

                    # evict + normalize: er_raw = o[:, :64] * (1/rowsum)
                    # (reciprocal also evicts the psum rowsums to sbuf so the
                    # multiply has a single psum operand)
                    o4 = o.rearrange("p (g s v) -> p g s v", s=2, v=VW)
                    rc = rpool.tile([BCAP, g * 2], fp32, tag="rc")
                    nc.vector.reciprocal(
                        out=rc.rearrange("p (g s v) -> p g s v", s=2, v=1),
                        in_=o4[:, :, :, D:D + 1])
                    er_raw = rpool.tile([BCAP, g * 2 * D], fp16, tag="eraw")
                    er4 = er_raw.rearrange("p (g s v) -> p g s v", s=2, v=D)
                    nc.vector.tensor_tensor(
                        out=er4, in0=o4[:, :, :, 0:D],
                        in1=rc.rearrange("p (g s v) -> p g s v", s=2, v=1)
                            .broadcast_to([BCAP, g, 2, D]),
                        op=ALU.mult)
                    # + x residual (sbuf only -> Pool engine)
                    er = rpool.tile([BCAP, g * 2 * D], fp16, tag="er")
                    rc = (blo + j0) * 2 * D
                    nc.gpsimd.tensor_tensor(
                        out=er, in0=er_raw,
                        in1=sb_vres[:, rc:rc + g * 2 * D], op=ALU.add)
                    # feature-major eviction straight into the MLP input
                    for j in range(j0, j0 + g):
                        c = (blo + j) * BCAP
                        jj = (j - j0) * BCAP
                        eng = nc.sync if (blo + j) % 2 == 0 else nc.scalar
                        eng.dma_start_transpose(
                            out=sb_eT[:, c:c + BCAP],
                            in_=er[:, jj:jj + BCAP])

            # ---- packed MLP over [128, rows] ----
            for c in range(0, rows, mlp_ch):
                hp = ps_m.tile([2 * D, mlp_ch], fp32, tag="m")
                nc.tensor.matmul(hp, sb_w1, sb_eT[:, c:c + mlp_ch],
                                 start=True, stop=True)
                nc.scalar.activation(out=sb_h[:, c:c + mlp_ch], in_=hp,
                                     func=AF.Relu, bias=sb_b1, scale=1.0)
            for c in range(0, rows, mlp_ch):
                op2 = ps_m.tile([2 * D, mlp_ch], fp32, tag="m")
                nc.tensor.matmul(op2, sb_w2, sb_h[:, c:c + mlp_ch],
                                 start=True, stop=True)
                nc.vector.scalar_tensor_tensor(
                    out=sb_out[:, c:c + mlp_ch], in0=op2, scalar=sb_b2,
                    in1=sb_eT[:, c:c + mlp_ch], op0=ALU.add, op1=ALU.add)
                nc.sync.dma_start(out=outp[:, c:c + mlp_ch],
                                  in_=sb_out[:, c:c + mlp_ch])

    nc.compile()
    return nc


def _pack_blocks(cnt_s, cnt_t):
    """Pair graphs into 128-row blocks. Returns list of blocks, each a
    list of (graph_id, row_offset)."""
    n = np.maximum(cnt_s, cnt_t)
    order = np.argsort(n, kind="stable")
    lo, hi = 0, len(order) - 1
    blocks = []
    while lo <= hi:
        g_hi = order[hi]
        if lo < hi and n[order[lo]] + n[g_hi] <= BCAP:
            g_lo = order[lo]
            blocks.append([(int(g_hi), 0), (int(g_lo), int(n[g_hi]))])
            lo += 1
        else:
            blocks.append([(int(g_hi), 0)])
        hi -= 1
    return blocks


def _shard_inputs(x_src, batch_src, x_tar, batch_tar, w1, b1, w2, b2):
    bs = np.asarray(batch_src).astype(np.int64)
    bt = np.asarray(batch_tar).astype(np.int64)
    xs = np.asarray(x_src, dtype=np.float32)
    xt = np.asarray(x_tar, dtype=np.float32)

    bnd_s = np.searchsorted(bs, np.arange(G + 1))
    bnd_t = np.searchsorted(bt, np.arange(G + 1))
    cnt_s = np.diff(bnd_s)
    cnt_t = np.diff(bnd_t)
    if np.maximum(cnt_s, cnt_t).max(initial=0) > BCAP:
        return None, None, (bnd_s, bnd_t, cnt_s, cnt_t)

    blocks = _pack_blocks(cnt_s, cnt_t)
    nb = (len(blocks) + N_CORES - 1) // N_CORES
    rows = nb * BCAP
    # deal blocks to cores round-robin
    core_blocks = [[] for _ in range(N_CORES)]
    for i, blk in enumerate(blocks):
        core_blocks[i % N_CORES].append(blk)

    w1a = np.asarray(w1, dtype=np.float32)
    w2a = np.asarray(w2, dtype=np.float32)
    b1a = np.asarray(b1, dtype=np.float32).reshape(D)
    b2a = np.asarray(b2, dtype=np.float32).reshape(D)
    w1bd = np.zeros((2 * D, 2 * D), dtype=np.float16)
    w2bd = np.zeros((2 * D, 2 * D), dtype=np.float16)
    w1bd[:D, :D] = w1a; w1bd[D:, D:] = w1a
    w2bd[:D, :D] = w2a; w2bd[D:, D:] = w2a
    b1bd = np.concatenate([b1a, b1a]).reshape(2 * D, 1)
    b2bd = np.concatenate([b2a, b2a]).reshape(2 * D, 1)

    to_bf16 = _bf16_caster()

    in_maps = []
    placement = []   # per core: list of (g, row_off_in_core_img)
    for c in range(N_CORES):
        blks = core_blocks[c]
        xs_img = np.zeros((CD, rows), dtype=np.float16)
        xt_img = np.zeros((CD, rows), dtype=np.float16)
        vres_img = np.zeros((BCAP, nb * 2 * D), dtype=np.float16)
        vs_img = np.zeros((BCAP, nb * VW), dtype=np.float32)
        vt_img = np.zeros((BCAP, nb * VW), dtype=np.float32)
        vs_img[:, VW - 1::VW] = 1.0   # mask column: ones everywhere
        vt_img[:, VW - 1::VW] = 1.0
        # ones bias row on ALL query rows (incl. padding): pad queries then
        # score -NEG against every key, so their exp(~0) rows don't pollute
        # the transposed-side rowsums through the all-ones mask column
        xs_img[D, :] = 1.0
        place = []
        for bi, blk in enumerate(blks):
            col = bi * BCAP
            for gi, (g, off) in enumerate(blk):
                ns, nt = cnt_s[g], cnt_t[g]
                sig = 1.0 if gi == 0 else -1.0
                xs_img[:D, col + off:col + off + ns] = xs[bnd_s[g]:bnd_s[g + 1]].T
                xt_img[:D, col + off:col + off + nt] = xt[bnd_t[g]:bnd_t[g + 1]].T
                # bias rows: score' = q.k + 1_q*(-NEG)_k + sig_q*(NEG*sig)_k
                xs_img[D + 1, col + off:col + off + ns] = sig
                xt_img[D, col + off:col + off + nt] = -NEG
                xt_img[D + 1, col + off:col + off + nt] = NEG * sig
                vs_img[off:off + ns, bi * VW:bi * VW + D] = xs[bnd_s[g]:bnd_s[g + 1]]
                vt_img[off:off + nt, bi * VW:bi * VW + D] = xt[bnd_t[g]:bnd_t[g + 1]]
                vres_img[off:off + ns, bi * 2 * D:bi * 2 * D + D] = \
                    xs[bnd_s[g]:bnd_s[g + 1]]
                vres_img[off:off + nt, bi * 2 * D + D:(bi + 1) * 2 * D] = \
                    xt[bnd_t[g]:bnd_t[g + 1]]
                place.append((int(g), col + off))
        # pad columns of real (non-empty) blocks still get the -NEG bias so
        # their exp is ~0; zero-filled xt bias rows already handle empty
        # blocks (rowsum = 128 from the ones mask -> no div by zero)
        for bi in range(len(blks)):
            col = bi * BCAP
            m = xt_img[D, col:col + BCAP] == 0.0
            xt_img[D, col:col + BCAP][m] = -NEG
        ident = np.eye(BCAP, dtype=np.float32)
        in_maps.append({
            "xsT": xs_img,
            "xtT": xt_img,
            "vs": to_bf16(vs_img),
            "vt": to_bf16(vt_img),
            "vres": vres_img,
            "idh": ident.astype(np.float16),
            "idb": to_bf16(ident),
            "w1bd": w1bd, "b1bd": b1bd, "w2bd": w2bd, "b2bd": b2bd,
        })
        placement.append(place)
    meta = (bnd_s, bnd_t, cnt_s, cnt_t, placement, nb)
    return in_maps, nb, meta


def _bf16_caster():
    import ml_dtypes
    return lambda a: a.astype(ml_dtypes.bfloat16)


def _numpy_fallback(x_src, batch_src, x_tar, batch_tar, w1, b1, w2, b2):
    bs = np.asarray(batch_src); bt = np.asarray(batch_tar)
    xs = np.asarray(x_src, dtype=np.float64); xt = np.asarray(x_tar, dtype=np.float64)
    mask = bs[:, None] == bt[None, :]

    def attend(q, kv, m):
        s = np.where(m, q @ kv.T, -1.0e9)
        s = s - s.max(axis=1, keepdims=True)
        e = np.exp(s)
        a = e / e.sum(axis=1, keepdims=True)
        out = a @ kv + q
        return np.where(m.any(axis=1, keepdims=True), out, 0.0)

    def mlp(x):
        return np.maximum(x @ w1 + b1, 0.0) @ w2 + b2 + x

    es = mlp(attend(xs, xt, mask))
    et = mlp(attend(xt, xs, mask.T))
    return et.astype(np.float32), es.astype(np.float32)


def kernel(x_src, batch_src, x_tar, batch_tar, w1, b1, w2, b2):
    in_maps, nb, meta = _shard_inputs(
        x_src, batch_src, x_tar, batch_tar, w1, b1, w2, b2)
    if in_maps is None:  # a graph overflowed BCAP; never happens for spec data
        return _numpy_fallback(
            x_src, batch_src, x_tar, batch_tar, w1, b1, w2, b2)
    bnd_s, bnd_t, cnt_s, cnt_t, placement, nb = meta
    rows = nb * BCAP
    mlp_ch = rows // 4
    assert rows % 4 == 0 and mlp_ch <= 512

    import os
    from concourse import bass_utils
    key = (nb, mlp_ch)
    if key not in _PROGRAM_CACHE:
        _PROGRAM_CACHE[key] = _build_program(nb, mlp_ch)
    nc = _PROGRAM_CACHE[key]
    trace = bool(os.environ.get("KERNEL_TRACE"))
    res = bass_utils.run_bass_kernel_spmd(
        nc, in_maps, core_ids=list(range(N_CORES)), trace=trace)
    _PROGRAM_CACHE["last_result"] = res

    # rows whose graph has no counterpart: reference yields mlp(0)
    w1a = np.asarray(w1, np.float32); b1a = np.asarray(b1, np.float32)
    w2a = np.asarray(w2, np.float32); b2a = np.asarray(b2, np.float32)
    mlp0 = np.maximum(b1a, 0.0) @ w2a + b2a

    embed_src = np.zeros((N_NODES, D), dtype=np.float32)
    embed_tar = np.zeros((N_NODES, D), dtype=np.float32)
    for c in range(N_CORES):
        op = np.asarray(res.results[c]["outp"]).astype(np.float32)
        for g, off in placement[c]:
            ns, nt = cnt_s[g], cnt_t[g]
            if ns > 0:
                embed_src[bnd_s[g]:bnd_s[g] + ns] = (
                    op[0:D, off:off + ns].T if nt > 0 else mlp0)
            if nt > 0:
                embed_tar[bnd_t[g]:bnd_t[g] + nt] = (
                    op[D:2 * D, off:off + nt].T if ns > 0 else mlp0)
    return embed_tar, embed_src


# revision 21
# speedup vs baseline: 1.0119x; 1.0119x over previous
"""Block-diagonal cross-attention + MLP for trn2, 8-core data-parallel.

v3: graphs bin-packed in pairs into 128-row blocks (NB blocks/core).
Cross-graph + padding masking is folded into the score matmul via two
extra contraction rows (graph-code sigma in {+1,-1}): score' = q.k
- 25*(1 - sigma_q*sigma_k), so cross-graph pairs get -50 and padded
columns -25 -> exp ~ 0.  Only exp(SS) is computed on ACT; E^T comes
from a DMA transpose (ST = SS^T exactly).  V-matmul eviction fuses
normalize (divide by the mask-column rowsum) and the +x residual via
scalar_tensor_tensor on DVE/Pool.  er is DMA-transposed straight into
the feature-major MLP input.  MLP uses block-diagonal [128,128]
weights to do both sides at once; residual+bias fused into the final
eviction.  dtypes: fp16 images/weights everywhere except E/V (bf16,
exp can reach e^46 which overflows fp16).
Output: [128, NB*128] fp16 per core; host scatters per graph.
"""

from contextlib import ExitStack

import numpy as np

N_NODES = 8192
D = 64
G = 128
N_CORES = 8
BCAP = 128                  # rows per block
CD = D + 2                  # contraction rows incl. bias rows
VW = D + 1                  # v image width incl. mask column
NEG = 25.0                  # pad bias; cross-graph pairs get -2*NEG

_PROGRAM_CACHE = {}


def _build_program(nb, mlp_ch):
    import concourse.bass as bass
    import concourse.tile as tile
    from concourse import bacc, mybir

    fp32 = mybir.dt.float32
    fp16 = mybir.dt.float16
    bf16 = mybir.dt.bfloat16
    rows = nb * BCAP
    nsb = (nb + 1) // 2           # superblocks of up to 2 blocks
    nc = bacc.Bacc("TRN2", target_bir_lowering=False, debug=False)

    xsT = nc.declare_dram_parameter("xsT", [CD, rows], fp16, isOutput=False)
    xtT = nc.declare_dram_parameter("xtT", [CD, rows], fp16, isOutput=False)
    vs = nc.declare_dram_parameter("vs", [BCAP, nb * VW], bf16, isOutput=False)
    vt = nc.declare_dram_parameter("vt", [BCAP, nb * VW], bf16, isOutput=False)
    vres = nc.declare_dram_parameter("vres", [BCAP, nb * 2 * D], fp16,
                                     isOutput=False)
    w1bd = nc.declare_dram_parameter("w1bd", [2 * D, 2 * D], fp16, isOutput=False)
    b1bd = nc.declare_dram_parameter("b1bd", [2 * D, 1], fp32, isOutput=False)
    w2bd = nc.declare_dram_parameter("w2bd", [2 * D, 2 * D], fp16, isOutput=False)
    b2bd = nc.declare_dram_parameter("b2bd", [2 * D, 1], fp32, isOutput=False)
    outp = nc.declare_dram_parameter("outp", [2 * D, rows], fp16, isOutput=True)

    AF = mybir.ActivationFunctionType
    ALU = mybir.AluOpType

    with tile.TileContext(nc) as tc, ExitStack() as ctx:
        singles = ctx.enter_context(tc.tile_pool(name="singles", bufs=1))
        epool = ctx.enter_context(tc.tile_pool(name="epool", bufs=3))
        tpool = ctx.enter_context(tc.tile_pool(name="tpool", bufs=3))
        rpool = ctx.enter_context(tc.tile_pool(name="rpool", bufs=3))

        sb_xsT = singles.tile([CD, rows], fp16, tag="xsT")
        sb_xtT = singles.tile([CD, rows], fp16, tag="xtT")
        sb_vs = singles.tile([BCAP, nb * VW], bf16, tag="vs")
        sb_vt = singles.tile([BCAP, nb * VW], bf16, tag="vt")
        sb_vres = singles.tile([BCAP, nb * 2 * D], fp16, tag="vres")
        sb_w1 = singles.tile([2 * D, 2 * D], fp16, tag="w1")
        sb_b1 = singles.tile([2 * D, 1], fp32, tag="b1")
        sb_w2 = singles.tile([2 * D, 2 * D], fp16, tag="w2")
        sb_b2 = singles.tile([2 * D, 1], fp32, tag="b2")
        sb_eT = singles.tile([2 * D, rows], fp16, tag="eT")
        sb_h = singles.tile([2 * D, rows], fp16, tag="h")
        sb_out = singles.tile([2 * D, rows], fp16, tag="out")

        warm = singles.tile([BCAP, 1], fp32, tag="warm")
        nc.vector.memset(warm, 0.0)
        nc.scalar.activation(out=warm, in_=warm, func=AF.Exp)
        sp = 2 * BCAP
        nc.sync.dma_start(out=sb_xsT[:, 0:sp], in_=xsT[:, 0:sp])
        nc.sync.dma_start(out=sb_xtT[:, 0:sp], in_=xtT[:, 0:sp])
        nc.sync.dma_start(out=sb_xsT[:, sp:], in_=xsT[:, sp:])
        nc.sync.dma_start(out=sb_xtT[:, sp:], in_=xtT[:, sp:])
        nc.scalar.dma_start(out=sb_vs, in_=vs[:, :])
        nc.scalar.dma_start(out=sb_vt, in_=vt[:, :])
        nc.scalar.dma_start(out=sb_vres, in_=vres[:, :])
        nc.sync.dma_start(out=sb_w1, in_=w1bd[:, :])
        nc.sync.dma_start(out=sb_b1, in_=b1bd[:, :])
        nc.sync.dma_start(out=sb_w2, in_=w2bd[:, :])
        nc.sync.dma_start(out=sb_b2, in_=b2bd[:, :])

        with tc.tile_pool(name="ps_sc", bufs=3, space="PSUM") as ps_sc, \
             tc.tile_pool(name="ps_ot", bufs=3, space="PSUM") as ps_ot, \
             tc.tile_pool(name="ps_m", bufs=2, space="PSUM") as ps_m:
            for s in range(nsb):
                blo = 2 * s
                g = min(2, nb - blo)
                # scores for g blocks, BOTH orientations [SS_j|ST_j] in one
                # psum bank -> one exp; ST computed directly on PE so no
                # E^T transpose is ever needed
                sc = ps_sc.tile([BCAP, g * 2 * BCAP], fp32, tag="sc")
                for j in range(g):
                    c = (blo + j) * BCAP
                    q = j * 2 * BCAP
                    nc.tensor.matmul(sc[:, q:q + BCAP],
                                     sb_xsT[:, c:c + BCAP],
                                     sb_xtT[:, c:c + BCAP],
                                     start=True, stop=True)
                    nc.tensor.matmul(sc[:, q + BCAP:q + 2 * BCAP],
                                     sb_xtT[:, c:c + BCAP],
                                     sb_xsT[:, c:c + BCAP],
                                     start=True, stop=True)
                et = epool.tile([BCAP, g * 2 * BCAP], bf16, tag="E")
                nc.scalar.activation(out=et, in_=sc, func=AF.Exp)

                if True:
                    j0 = 0
                    o = ps_ot.tile([BCAP, g * 2 * VW], fp32, tag="ot")
                    for j in range(j0, j0 + g):
                        b = blo + j
                        va = b * VW
                        q = (j - j0) * 2 * VW
                        e_ss = et[:, j * 2 * BCAP:j * 2 * BCAP + BCAP]
                        e_st = et[:, j * 2 * BCAP + BCAP:(j + 1) * 2 * BCAP]
                        nc.tensor.matmul(o[:, q:q + VW], e_st,
                                         sb_vt[:, va:va + VW],
                                         start=True, stop=True)
                        nc.tensor.matmul(o[:, q + VW:q + 2 * VW], e_ss,
                                         sb_vs[:, va:va + VW],
                                         start=True, stop=True)

                    # evict + normalize: er_raw = o[:, :64] * (1/rowsum)
                    # (reciprocal also evicts the psum rowsums to sbuf so the
                    # multiply has a single psum operand)
                    o4 = o.rearrange("p (g s v) -> p g s v", s=2, v=VW)
                    rc = rpool.tile([BCAP, g * 2], fp32, tag="rc")
                    nc.vector.reciprocal(
                        out=rc.rearrange("p (g s v) -> p g s v", s=2, v=1),
                        in_=o4[:, :, :, D:D + 1])
                    er_raw = rpool.tile([BCAP, g * 2 * D], fp16, tag="eraw")
                    er4 = er_raw.rearrange("p (g s v) -> p g s v", s=2, v=D)
                    nc.vector.tensor_tensor(
                        out=er4, in0=o4[:, :, :, 0:D],
                        in1=rc.rearrange("p (g s v) -> p g s v", s=2, v=1)
                            .broadcast_to([BCAP, g, 2, D]),
                        op=ALU.mult)
                    # + x residual (sbuf only -> Pool engine)
                    er = rpool.tile([BCAP, g * 2 * D], fp16, tag="er")
                    rc = (blo + j0) * 2 * D
                    nc.gpsimd.tensor_tensor(
                        out=er, in0=er_raw,
                        in1=sb_vres[:, rc:rc + g * 2 * D], op=ALU.add)
                    # feature-major eviction straight into the MLP input
                    for j in range(j0, j0 + g):
                        c = (blo + j) * BCAP
                        jj = (j - j0) * BCAP
                        eng = nc.sync if (blo + j) % 2 == 0 else nc.scalar
                        eng.dma_start_transpose(
                            out=sb_eT[:, c:c + BCAP],
                            in_=er[:, jj:jj + BCAP])

            # ---- packed MLP over [128, rows] ----
            for c in range(0, rows, mlp_ch):
                hp = ps_m.tile([2 * D, mlp_ch], fp32, tag="m")
                nc.tensor.matmul(hp, sb_w1, sb_eT[:, c:c + mlp_ch],
                                 start=True, stop=True)
                nc.scalar.activation(out=sb_h[:, c:c + mlp_ch], in_=hp,
                                     func=AF.Relu, bias=sb_b1, scale=1.0)
            for c in range(0, rows, mlp_ch):
                op2 = ps_m.tile([2 * D, mlp_ch], fp32, tag="m")
                nc.tensor.matmul(op2, sb_w2, sb_h[:, c:c + mlp_ch],
                                 start=True, stop=True)
                nc.vector.scalar_tensor_tensor(
                    out=sb_out[:, c:c + mlp_ch], in0=op2, scalar=sb_b2,
                    in1=sb_eT[:, c:c + mlp_ch], op0=ALU.add, op1=ALU.add)
                nc.sync.dma_start(out=outp[:, c:c + mlp_ch],
                                  in_=sb_out[:, c:c + mlp_ch])

    nc.compile()
    return nc


def _pack_blocks(cnt_s, cnt_t):
    """Pair graphs into 128-row blocks. Returns list of blocks, each a
    list of (graph_id, row_offset)."""
    n = np.maximum(cnt_s, cnt_t)
    order = np.argsort(n, kind="stable")
    lo, hi = 0, len(order) - 1
    blocks = []
    while lo <= hi:
        g_hi = order[hi]
        if lo < hi and n[order[lo]] + n[g_hi] <= BCAP:
            g_lo = order[lo]
            blocks.append([(int(g_hi), 0), (int(g_lo), int(n[g_hi]))])
            lo += 1
        else:
            blocks.append([(int(g_hi), 0)])
        hi -= 1
    return blocks


def _shard_inputs(x_src, batch_src, x_tar, batch_tar, w1, b1, w2, b2):
    bs = np.asarray(batch_src).astype(np.int64)
    bt = np.asarray(batch_tar).astype(np.int64)
    xs = np.asarray(x_src, dtype=np.float32)
    xt = np.asarray(x_tar, dtype=np.float32)

    bnd_s = np.searchsorted(bs, np.arange(G + 1))
    bnd_t = np.searchsorted(bt, np.arange(G + 1))
    cnt_s = np.diff(bnd_s)
    cnt_t = np.diff(bnd_t)
    if np.maximum(cnt_s, cnt_t).max(initial=0) > BCAP:
        return None, None, (bnd_s, bnd_t, cnt_s, cnt_t)

    blocks = _pack_blocks(cnt_s, cnt_t)
    nb = (len(blocks) + N_CORES - 1) // N_CORES
    rows = nb * BCAP
    # deal blocks to cores round-robin
    core_blocks = [[] for _ in range(N_CORES)]
    for i, blk in enumerate(blocks):
        core_blocks[i % N_CORES].append(blk)

    w1a = np.asarray(w1, dtype=np.float32)
    w2a = np.asarray(w2, dtype=np.float32)
    b1a = np.asarray(b1, dtype=np.float32).reshape(D)
    b2a = np.asarray(b2, dtype=np.float32).reshape(D)
    w1bd = np.zeros((2 * D, 2 * D), dtype=np.float16)
    w2bd = np.zeros((2 * D, 2 * D), dtype=np.float16)
    w1bd[:D, :D] = w1a; w1bd[D:, D:] = w1a
    w2bd[:D, :D] = w2a; w2bd[D:, D:] = w2a
    b1bd = np.concatenate([b1a, b1a]).reshape(2 * D, 1)
    b2bd = np.concatenate([b2a, b2a]).reshape(2 * D, 1)

    to_bf16 = _bf16_caster()

    in_maps = []
    placement = []   # per core: list of (g, row_off_in_core_img)
    for c in range(N_CORES):
        blks = core_blocks[c]
        xs_img = np.zeros((CD, rows), dtype=np.float16)
        xt_img = np.zeros((CD, rows), dtype=np.float16)
        vres_img = np.zeros((BCAP, nb * 2 * D), dtype=np.float16)
        vs_img = np.zeros((BCAP, nb * VW), dtype=np.float32)
        vt_img = np.zeros((BCAP, nb * VW), dtype=np.float32)
        vs_img[:, VW - 1::VW] = 1.0   # mask column: ones everywhere
        vt_img[:, VW - 1::VW] = 1.0
        # ones bias row on ALL query rows (incl. padding): pad queries then
        # score -NEG against every key, so their exp(~0) rows don't pollute
        # the transposed-side rowsums through the all-ones mask column
        xs_img[D, :] = 1.0
        place = []
        for bi, blk in enumerate(blks):
            col = bi * BCAP
            for gi, (g, off) in enumerate(blk):
                ns, nt = cnt_s[g], cnt_t[g]
                sig = 1.0 if gi == 0 else -1.0
                xs_img[:D, col + off:col + off + ns] = xs[bnd_s[g]:bnd_s[g + 1]].T
                xt_img[:D, col + off:col + off + nt] = xt[bnd_t[g]:bnd_t[g + 1]].T
                # bias rows: score' = q.k + 1_q*(-NEG)_k + sig_q*(NEG*sig)_k
                xs_img[D + 1, col + off:col + off + ns] = sig
                xt_img[D, col + off:col + off + nt] = -NEG
                xt_img[D + 1, col + off:col + off + nt] = NEG * sig
                vs_img[off:off + ns, bi * VW:bi * VW + D] = xs[bnd_s[g]:bnd_s[g + 1]]
                vt_img[off:off + nt, bi * VW:bi * VW + D] = xt[bnd_t[g]:bnd_t[g + 1]]
                vres_img[off:off + ns, bi * 2 * D:bi * 2 * D + D] = \
                    xs[bnd_s[g]:bnd_s[g + 1]]
                vres_img[off:off + nt, bi * 2 * D + D:(bi + 1) * 2 * D] = \
                    xt[bnd_t[g]:bnd_t[g + 1]]
                place.append((int(g), col + off))
        # pad columns of real (non-empty) blocks still get the -NEG bias so
        # their exp is ~0; zero-filled xt bias rows already handle empty
        # blocks (rowsum = 128 from the ones mask -> no div by zero)
        for bi in range(len(blks)):
            col = bi * BCAP
            m = xt_img[D, col:col + BCAP] == 0.0
            xt_img[D, col:col + BCAP][m] = -NEG
        in_maps.append({
            "xsT": xs_img,
            "xtT": xt_img,
            "vs": to_bf16(vs_img),
            "vt": to_bf16(vt_img),
            "vres": vres_img,
            "w1bd": w1bd, "b1bd": b1bd, "w2bd": w2bd, "b2bd": b2bd,
        })
        placement.append(place)
    meta = (bnd_s, bnd_t, cnt_s, cnt_t, placement, nb)
    return in_maps, nb, meta


def _bf16_caster():
    import ml_dtypes
    return lambda a: a.astype(ml_dtypes.bfloat16)


def _numpy_fallback(x_src, batch_src, x_tar, batch_tar, w1, b1, w2, b2):
    bs = np.asarray(batch_src); bt = np.asarray(batch_tar)
    xs = np.asarray(x_src, dtype=np.float64); xt = np.asarray(x_tar, dtype=np.float64)
    mask = bs[:, None] == bt[None, :]

    def attend(q, kv, m):
        s = np.where(m, q @ kv.T, -1.0e9)
        s = s - s.max(axis=1, keepdims=True)
        e = np.exp(s)
        a = e / e.sum(axis=1, keepdims=True)
        out = a @ kv + q
        return np.where(m.any(axis=1, keepdims=True), out, 0.0)

    def mlp(x):
        return np.maximum(x @ w1 + b1, 0.0) @ w2 + b2 + x

    es = mlp(attend(xs, xt, mask))
    et = mlp(attend(xt, xs, mask.T))
    return et.astype(np.float32), es.astype(np.float32)


def kernel(x_src, batch_src, x_tar, batch_tar, w1, b1, w2, b2):
    in_maps, nb, meta = _shard_inputs(
        x_src, batch_src, x_tar, batch_tar, w1, b1, w2, b2)
    if in_maps is None:  # a graph overflowed BCAP; never happens for spec data
        return _numpy_fallback(
            x_src, batch_src, x_tar, batch_tar, w1, b1, w2, b2)
    bnd_s, bnd_t, cnt_s, cnt_t, placement, nb = meta
    rows = nb * BCAP
    mlp_ch = rows // 4
    assert rows % 4 == 0 and mlp_ch <= 512

    import os
    from concourse import bass_utils
    key = (nb, mlp_ch)
    if key not in _PROGRAM_CACHE:
        _PROGRAM_CACHE[key] = _build_program(nb, mlp_ch)
    nc = _PROGRAM_CACHE[key]
    trace = bool(os.environ.get("KERNEL_TRACE"))
    res = bass_utils.run_bass_kernel_spmd(
        nc, in_maps, core_ids=list(range(N_CORES)), trace=trace)
    _PROGRAM_CACHE["last_result"] = res

    # rows whose graph has no counterpart: reference yields mlp(0)
    w1a = np.asarray(w1, np.float32); b1a = np.asarray(b1, np.float32)
    w2a = np.asarray(w2, np.float32); b2a = np.asarray(b2, np.float32)
    mlp0 = np.maximum(b1a, 0.0) @ w2a + b2a

    embed_src = np.zeros((N_NODES, D), dtype=np.float32)
    embed_tar = np.zeros((N_NODES, D), dtype=np.float32)
    for c in range(N_CORES):
        op = np.asarray(res.results[c]["outp"]).astype(np.float32)
        for g, off in placement[c]:
            ns, nt = cnt_s[g], cnt_t[g]
            if ns > 0:
                embed_src[bnd_s[g]:bnd_s[g] + ns] = (
                    op[0:D, off:off + ns].T if nt > 0 else mlp0)
            if nt > 0:
                embed_tar[bnd_t[g]:bnd_t[g] + nt] = (
                    op[D:2 * D, off:off + nt].T if ns > 0 else mlp0)
    return embed_tar, embed_src
